# revision 1
# baseline (speedup 1.0000x reference)
"""ConvLRU model kernel for 8 Trainium2 NeuronCores (Bass/Tile).

Sharding: width-parallel. Core k owns output columns [8k, 8k+8) of every frame.
 - conv feature extraction: frame-pair stacked [128 = 2 frames x 64ch, 512 pos]
   tiles, 49 tap matmuls accumulating in PSUM (fp32r, block-diag weights).
 - GroupNorm: per-core partial sums (fused into PSUM-evac / square passes via
   accum_out), one 16KB AllReduce across the 8 cores, stats finished redundantly.
 - gated pointwise convs + residual: block-diag paired matmuls + ACT/DVE ops.
 - LRU over L: y_l = sum_j G[l,j,c] u_j with per-channel lower-triangular G
   matrices precomputed on host from dt/nu/theta/c/d -> pure matmuls on device.

kernel(**inputs) takes FULL inputs, returns FULL [B, C, L, H, W] output.
"""
import os
import sys

if os.environ.get("JAX_PLATFORMS") == "cpu":
    # the bass kernel can only execute on the neuron cores
    os.environ["JAX_PLATFORMS"] = "axon"

try:
    import concourse.bass  # noqa: F401
except ImportError:
    sys.path.insert(0, "/opt/trn_rl_repo")

import numpy as np
import concourse.bacc as bacc
import concourse.tile as tile
from concourse import mybir
from concourse.bass_utils import run_bass_kernel_spmd

dt_ = mybir.dt

B, C, L, H, W = 2, 64, 16, 64, 64
KK = 7
PAD = 3
NCORES = 8
WSL = W // NCORES            # 8 output cols per core
WPAD = WSL + 2 * PAD         # 14
HP = H + 2 * PAD             # 70
NPP = L // 2                 # 8 l-pairs per batch
NPAIRS = B * NPP             # 16 pair tiles
SPOS = H * WSL               # 512 positions per frame-slice
NTAPS = KK * KK              # 49
NGM = B * (NPP * (NPP + 1) // 2)  # 72 LRU matrices
NELEM = 16 * H * W           # groupnorm group element count


# ---------------------------------------------------------------- host prep

def _host_prep(inputs):
    x = np.asarray(inputs["x"], np.float32)
    W_sp = np.asarray(inputs["W_sp"], np.float32)
    W_dc = np.asarray(inputs["W_dc"], np.float32)
    W_in = np.asarray(inputs["W_in"], np.float32)
    W_out = np.asarray(inputs["W_out"], np.float32)
    b_dc = np.asarray(inputs["b_dc"], np.float32)
    b_in = np.asarray(inputs["b_in"], np.float32)
    b_out = np.asarray(inputs["b_out"], np.float32)
    gn_g = np.asarray(inputs["gn_g"], np.float32)
    gn_b = np.asarray(inputs["gn_b"], np.float32)

    xf = x.transpose(0, 2, 1, 3, 4).reshape(B * L, C, H, W)
    xp = np.pad(xf, ((0, 0), (0, 0), (0, 0), (PAD, PAD)), mode="wrap")
    xp = np.pad(xp, ((0, 0), (0, 0), (PAD, PAD), (0, 0)), mode="edge")  # [BL,C,70,70]

    xin = []
    for k in range(NCORES):
        sl = xp[:, :, :, WSL * k : WSL * k + WPAD]              # [32, 64, 70, 14]
        t = sl.reshape(NPAIRS, 2 * C, HP * WPAD)                # pairs stacked
        xin.append(np.ascontiguousarray(t))

    W_eff = np.einsum("oc,cixy->oixy", W_dc, W_sp)
    wconv = np.zeros((NTAPS, 2 * C, 2 * C), np.float32)
    for t in range(NTAPS):
        ky, kx = divmod(t, KK)
        blk = np.ascontiguousarray(W_eff[:, :, ky, kx].T)
        wconv[t, :C, :C] = blk
        wconv[t, C:, C:] = blk
    # DRAM layout [k-partition, tap, m]
    wconv_d = np.ascontiguousarray(wconv.transpose(1, 0, 2))     # [128, 49, 128]

    win1 = np.zeros((2 * C, 2 * C), np.float32)
    win2 = np.zeros((2 * C, 2 * C), np.float32)
    win1[:C, :C] = W_in[:C].T
    win1[C:, C:] = W_in[:C].T
    win2[:C, :C] = W_in[C:].T
    win2[C:, C:] = W_in[C:].T
    woutm = np.zeros((2 * C, 2 * C), np.float32)
    woutm[:C, :C] = W_out.T
    woutm[C:, C:] = W_out.T

    gsel = np.zeros((2 * C, 2 * C), np.float32)
    grp = np.arange(C) // 16
    same = grp[:, None] == grp[None, :]
    gsel[:C, :C] = same
    gsel[C:, C:] = same

    vecs = np.zeros((2 * C, 6), np.float32)
    vecs[:, 0] = np.concatenate([b_dc, b_dc])
    vecs[:, 1] = np.concatenate([b_in[:C], b_in[:C]])
    vecs[:, 2] = np.concatenate([b_in[C:], b_in[C:]])
    vecs[:, 3] = np.concatenate([b_out, b_out])
    vecs[:, 4] = np.concatenate([gn_g, gn_g])
    vecs[:, 5] = np.concatenate([gn_b, gn_b])

    return xin, wconv_d, win1, win2, woutm, gsel, vecs


def _host_lru_g(inputs):
    dt = np.asarray(inputs["dt"], np.float64)
    nu = np.exp(np.asarray(inputs["nu_log"], np.float64))
    th = np.exp(np.asarray(inputs["theta_log"], np.float64))
    cr = np.asarray(inputs["c_re"], np.float64)
    ci = np.asarray(inputs["c_im"], np.float64)
    ds = np.asarray(inputs["d_skip"], np.float64)

    decay = np.exp(-nu[None, None, :] * dt[:, :, None])
    phase = th[None, None, :] * dt[:, :, None]
    lam = decay * np.exp(1j * phase)
    gam = np.sqrt(np.maximum(1.0 - decay**2, 1e-6))

    G = np.zeros((B, L, L, C), np.float64)
    for b in range(B):
        for l in range(L):
            P = np.ones(C, np.complex128)
            for j in range(l, -1, -1):
                Pg = P * gam[b, j]
                G[b, l, j] = cr * Pg.real + ci * Pg.imag
                P = P * lam[b, j]
        for l in range(L):
            G[b, l, l] += ds

    gm = np.zeros((NGM, 2 * C, 2 * C), np.float32)
    idx = 0
    for b in range(B):
        for p in range(NPP):
            for q in range(p + 1):
                m = np.zeros((2 * C, 2 * C), np.float64)
                for pl in range(2):
                    for pj in range(2):
                        l, j = 2 * p + pl, 2 * q + pj
                        if j <= l:
                            m[pj * C : pj * C + C, pl * C : pl * C + C] = np.diag(
                                G[b, l, j]
                            )
                gm[idx] = m
                idx += 1
    return np.ascontiguousarray(gm.transpose(1, 0, 2))           # [128, 72, 128]


# ---------------------------------------------------------------- bass build

_NC_CACHE = {}


def _build_nc():
    if "nc" in _NC_CACHE:
        return _NC_CACHE["nc"]
    f32, f32r = dt_.float32, dt_.float32r
    Act = mybir.ActivationFunctionType
    Alu = mybir.AluOpType

    nc = bacc.Bacc("TRN2", target_bir_lowering=False, debug=False)

    xin_d = nc.dram_tensor("xin", [NPAIRS, 2 * C, HP * WPAD], f32r, kind="ExternalInput")
    wconv_d = nc.dram_tensor("wconv", [2 * C, NTAPS, 2 * C], f32r, kind="ExternalInput")
    gm_d = nc.dram_tensor("gm", [2 * C, NGM, 2 * C], f32r, kind="ExternalInput")
    win1_d = nc.dram_tensor("win1", [2 * C, 2 * C], f32r, kind="ExternalInput")
    win2_d = nc.dram_tensor("win2", [2 * C, 2 * C], f32r, kind="ExternalInput")
    wout_d = nc.dram_tensor("wout", [2 * C, 2 * C], f32r, kind="ExternalInput")
    gsel_d = nc.dram_tensor("gsel", [2 * C, 2 * C], f32, kind="ExternalInput")
    vecs_d = nc.dram_tensor("vecs", [2 * C, 6], f32, kind="ExternalInput")
    yout_d = nc.dram_tensor("yout", [B, L, C, H, WSL], f32, kind="ExternalOutput")

    with tile.TileContext(nc) as tc:
        with (
            tc.tile_pool(name="wpool", bufs=1) as wpool,
            tc.tile_pool(name="xpool", bufs=1) as xpool,
            tc.tile_pool(name="ypool", bufs=1) as ypool,
            tc.tile_pool(name="upool", bufs=1) as upool,
            tc.tile_pool(name="spool", bufs=1) as spool,
            tc.tile_pool(name="tpool", bufs=3) as tpool,
            tc.tile_pool(name="opool", bufs=4) as opool,
            tc.tile_pool(name="gpool", bufs=12) as gpool,
            tc.tile_pool(name="dram", bufs=1, space="DRAM") as dram,
        ):
            # ---- inputs first (the first conv group needs xin[0..1] + taps)
            WCCH = [(0, 13), (13, 12), (25, 12), (37, 12)]
            xts = [None] * NPAIRS

            def _load_x(pr):
                xt = xpool.tile(
                    [2 * C, HP, WPAD], f32r, tag=f"x{pr}", name=f"x{pr}"
                )
                nc.sync.dma_start(out=xt[:], in_=xin_d[pr])
                xts[pr] = xt

            _load_x(0)
            _load_x(1)
            wconv_ts = []
            for ci, (t0, nt) in enumerate(WCCH):
                wct = wpool.tile(
                    [2 * C, nt, 2 * C], f32r, tag=f"wc{ci}", name=f"wc{ci}"
                )
                nc.sync.dma_start(out=wct[:], in_=wconv_d[:, t0 : t0 + nt, :])
                wconv_ts.append(wct)
                if ci == 0:
                    for pr in range(2, 8):
                        _load_x(pr)
            for pr in range(8, NPAIRS):
                _load_x(pr)

            def _wconv_ap(t):
                for ci, (t0, nt) in enumerate(WCCH):
                    if t0 <= t < t0 + nt:
                        return wconv_ts[ci][:, t - t0, :]
                raise AssertionError(t)

            win1_t = wpool.tile([2 * C, 2 * C], f32r, tag="win1")
            nc.sync.dma_start(out=win1_t[:], in_=win1_d[:])
            win2_t = wpool.tile([2 * C, 2 * C], f32r, tag="win2")
            nc.sync.dma_start(out=win2_t[:], in_=win2_d[:])
            wout_t = wpool.tile([2 * C, 2 * C], f32r, tag="wout")
            nc.sync.dma_start(out=wout_t[:], in_=wout_d[:])
            gsel_t = wpool.tile([2 * C, 2 * C], f32, tag="gsel")
            nc.sync.dma_start(out=gsel_t[:], in_=gsel_d[:])
            vecs_t = wpool.tile([2 * C, 6], f32, tag="vecs")
            nc.sync.dma_start(out=vecs_t[:], in_=vecs_d[:])
            eps_t = wpool.tile([2 * C, 1], f32, tag="eps")
            nc.vector.memset(eps_t[:], 1e-5)

            NQ = 4  # stats quarters: 4 pairs each, 4 pipelined AllGathers
            stats_t = [
                spool.tile([2 * C, 8], f32, tag=f"st{q}", name=f"st{q}")
                for q in range(NQ)
            ]
            cc_in = [
                dram.tile([2 * C, 8], f32, tag=f"ccin{q}", name=f"ccin{q}")
                for q in range(NQ)
            ]
            cc_out = [
                dram.tile([NCORES * 2 * C, 8], f32, tag=f"ccout{q}", name=f"ccout{q}")
                for q in range(NQ)
            ]
            statsr = [None] * NQ
            coeffs = [None] * NQ
            yts = [None] * NPAIRS
            uts = [None] * NPAIRS

            # cc/reduce path rides the gpsimd DMA queue so it never blocks
            # (or is blocked by) the bulk loads on the sync queue.
            def _launch_ag(q):
                nc.gpsimd.dma_start(out=cc_in[q][:], in_=stats_t[q][:])
                nc.gpsimd.collective_compute(
                    "AllGather",
                    mybir.AluOpType.bypass,
                    replica_groups=[list(range(NCORES))],
                    ins=[cc_in[q].opt()],
                    outs=[cc_out[q].opt()],
                )
                red = spool.tile(
                    [2 * C, NCORES, 8], f32, tag=f"red{q}", name=f"red{q}"
                )
                nc.gpsimd.dma_start(
                    out=red[:],
                    in_=cc_out[q][:].rearrange("(r p) f -> p r f", p=2 * C),
                )
                sr = spool.tile([2 * C, 8], f32, tag=f"sr{q}", name=f"sr{q}")
                nc.vector.tensor_reduce(
                    out=sr[:],
                    in_=red[:].rearrange("p r f -> p f r"),
                    axis=mybir.AxisListType.X,
                    op=Alu.add,
                )
                statsr[q] = sr

            def _stats_finish(q, psum_pool):
                sg = psum_pool.tile([2 * C, 8], f32, tag="sg", name=f"sg{q}")
                nc.tensor.matmul(sg[:], gsel_t[:], statsr[q][:], start=True, stop=True)
                mu_t = spool.tile([2 * C, 4], f32, tag=f"mu{q}", name=f"mu{q}")
                nc.scalar.activation(
                    out=mu_t[:], in_=sg[:, :4], func=Act.Copy, scale=1.0 / NELEM
                )
                m2_t = spool.tile([2 * C, 4], f32, tag=f"m2{q}", name=f"m2{q}")
                nc.scalar.activation(
                    out=m2_t[:], in_=sg[:, 4:], func=Act.Copy, scale=1.0 / NELEM
                )
                var_t = spool.tile([2 * C, 4], f32, tag=f"var{q}", name=f"var{q}")
                nc.vector.tensor_mul(var_t[:], mu_t[:], mu_t[:])
                nc.vector.tensor_sub(var_t[:], m2_t[:], var_t[:])
                std_t = spool.tile([2 * C, 4], f32, tag=f"std{q}", name=f"std{q}")
                nc.scalar.activation(
                    out=std_t[:], in_=var_t[:], func=Act.Sqrt, bias=eps_t[:], scale=1.0
                )
                a_t = spool.tile([2 * C, 4], f32, tag=f"aaff{q}", name=f"aaff{q}")
                nc.vector.reciprocal(a_t[:], std_t[:])
                nc.vector.tensor_scalar_mul(a_t[:], a_t[:], vecs_t[:, 4:5])
                b_t = spool.tile([2 * C, 4], f32, tag=f"baff{q}", name=f"baff{q}")
                nc.vector.tensor_mul(b_t[:], mu_t[:], a_t[:])
                nc.vector.tensor_scalar(
                    out=b_t[:],
                    in0=b_t[:],
                    scalar1=-1.0,
                    scalar2=vecs_t[:, 5:6],
                    op0=Alu.mult,
                    op1=Alu.add,
                )
                coeffs[q] = (a_t, b_t)

            def _gn_apply(pr):
                q, pq = divmod(pr, 4)
                a_t, b_t = coeffs[q]
                nc.scalar.activation(
                    out=yts[pr][:],
                    in_=yts[pr][:],
                    func=Act.Identity,
                    bias=b_t[:, pq : pq + 1],
                    scale=a_t[:, pq : pq + 1],
                )

            def _pair_chain(pr, wpsum, opsum):
                yt = yts[pr]
                h1 = wpsum.tile([2 * C, SPOS], f32, tag="h", name=f"h1_{pr}")
                nc.tensor.matmul(h1[:], win1_t[:], yt[:], start=True, stop=True)
                h2 = wpsum.tile([2 * C, SPOS], f32, tag="h", name=f"h2_{pr}")
                nc.tensor.matmul(h2[:], win2_t[:], yt[:], start=True, stop=True)
                sig = tpool.tile([2 * C, SPOS], f32, tag="sig", name=f"sig{pr}")
                nc.scalar.activation(
                    out=sig[:],
                    in_=h2[:],
                    func=Act.Sigmoid,
                    bias=vecs_t[:, 2:3],
                    scale=1.0,
                )
                zt = tpool.tile([2 * C, SPOS], f32r, tag="z", name=f"z{pr}")
                nc.vector.scalar_tensor_tensor(
                    out=zt[:],
                    in0=h1[:],
                    scalar=vecs_t[:, 1:2],
                    in1=sig[:],
                    op0=Alu.add,
                    op1=Alu.mult,
                )
                z2 = opsum.tile([2 * C, SPOS], f32, tag="z2", name=f"z2_{pr}")
                nc.tensor.matmul(z2[:], wout_t[:], zt[:], start=True, stop=True)
                ut = upool.tile([2 * C, SPOS], f32r, tag=f"u{pr}", name=f"u{pr}")
                ctr = xts[pr][:, PAD : PAD + H, PAD : PAD + WSL]
                nc.vector.scalar_tensor_tensor(
                    out=ut[:],
                    in0=z2[:],
                    scalar=vecs_t[:, 3:4],
                    in1=ctr,
                    op0=Alu.add,
                    op1=Alu.add,
                )
                uts[pr] = ut

            def _lru_row(b, p, lpsum):
                gidx_base = b * (NPP * (NPP + 1) // 2)
                lp = lpsum.tile([2 * C, SPOS], f32, tag="lp", name=f"lp{b}_{p}")
                for q in range(p + 1):
                    gidx = gidx_base + p * (p + 1) // 2 + q
                    gt = gpool.tile([2 * C, 2 * C], f32r, tag="g", name=f"g{gidx}")
                    nc.sync.dma_start(out=gt[:], in_=gm_d[:, gidx, :])
                    nc.tensor.matmul(
                        lp[:],
                        gt[:],
                        uts[b * NPP + q][:],
                        start=(q == 0),
                        stop=(q == p),
                    )
                yo = opool.tile([2 * C, SPOS], f32, tag="yo", name=f"yo{b}_{p}")
                nc.scalar.activation(out=yo[:], in_=lp[:], func=Act.Copy)
                nc.sync.dma_start(out=yout_d[b, 2 * p : 2 * p + 2], in_=yo[:])

            # ---- conv phase: tap-outer over pair groups. Quarter-stats
            # AllGathers launch as each quarter's pairs finish; q0/q1 finishes
            # and their GN-applies slot mid-conv (inputs ready by the time the
            # in-order PE/ACT queues reach them -> no stall).
            GROUPS = [
                range(0, 2), range(2, 4), range(4, 8),
                range(8, 12), range(12, 14), range(14, 16),
            ]
            with (
                tc.tile_pool(name="cpsum", bufs=6, space="PSUM") as cpsum,
                tc.tile_pool(name="s01psum", bufs=1, space="PSUM") as s01psum,
            ):
                for grp in GROUPS:
                    if grp.start == 12:
                        _stats_finish(0, s01psum)
                        for pr in range(0, 4):
                            _gn_apply(pr)
                    if grp.start == 14:
                        _stats_finish(1, s01psum)
                        for pr in range(4, 8):
                            _gn_apply(pr)
                    pss = {}
                    for pr in grp:
                        pss[pr] = cpsum.tile(
                            [2 * C, SPOS], f32, tag="cps", name=f"cps{pr}"
                        )
                    for t in range(NTAPS):
                        ky, kx = divmod(t, KK)
                        wap = _wconv_ap(t)
                        for pr in grp:
                            mov = xts[pr][:, ky : ky + H, kx : kx + WSL]
                            nc.tensor.matmul(
                                pss[pr][:],
                                wap,
                                mov,
                                start=(t == 0),
                                stop=(t == NTAPS - 1),
                            )
                    for pr in grp:
                        q, pq = divmod(pr, 4)
                        yt = ypool.tile(
                            [2 * C, SPOS], f32r, tag=f"y{pr}", name=f"y{pr}"
                        )
                        nc.scalar.activation(
                            out=yt[:],
                            in_=pss[pr][:],
                            func=Act.Identity,
                            bias=vecs_t[:, 0:1],
                            scale=1.0,
                            accum_out=stats_t[q][:, pq : pq + 1],
                        )
                        trash = tpool.tile([2 * C, SPOS], f32, tag="trash")
                        nc.vector.scalar_tensor_tensor(
                            out=trash[:],
                            in0=yt[:],
                            scalar=1.0,
                            in1=yt[:],
                            op0=Alu.bypass,
                            op1=Alu.mult,
                            accum_out=stats_t[q][:, 4 + pq : 4 + pq + 1],
                        )
                        yts[pr] = yt
                    if grp.stop in (4, 8, 12, 16):
                        _launch_ag(grp.stop // 4 - 1)

            with (
                tc.tile_pool(name="s23psum", bufs=1, space="PSUM") as s23psum,
                tc.tile_pool(name="wpsum", bufs=3, space="PSUM") as wpsum,
                tc.tile_pool(name="opsum", bufs=2, space="PSUM") as opsum,
                tc.tile_pool(name="lpsum", bufs=2, space="PSUM") as lpsum,
            ):
                # b0 chains (GN done in conv), then q2 stats, then b0 LRU rows
                # as ready PE filler, then q3 quarter, keeping evacs out of the
                # chains' ACT path to avoid head-of-line serialization.
                for p in range(0, 8):
                    _pair_chain(p, wpsum, opsum)
                _stats_finish(2, s23psum)
                for pr in range(8, 12):
                    _gn_apply(pr)
                # b0 LRU rows 0-4 while q2 coeffs resolve
                for p in range(0, 5):
                    _lru_row(0, p, lpsum)
                # chains 8-11 with ready b0 rows 5-7 as PE filler between the
                # sigmoid/gate round-trips
                _pair_chain(8, wpsum, opsum)
                _lru_row(0, 5, lpsum)
                _pair_chain(9, wpsum, opsum)
                _lru_row(0, 6, lpsum)
                _pair_chain(10, wpsum, opsum)
                _lru_row(0, 7, lpsum)
                _pair_chain(11, wpsum, opsum)
                _stats_finish(3, s23psum)
                for pr in range(12, 16):
                    _gn_apply(pr)
                for p in range(0, 2):
                    _lru_row(1, p, lpsum)
                _pair_chain(12, wpsum, opsum)
                _lru_row(1, 2, lpsum)
                _pair_chain(13, wpsum, opsum)
                _lru_row(1, 3, lpsum)
                _pair_chain(14, wpsum, opsum)
                _lru_row(1, 4, lpsum)
                _pair_chain(15, wpsum, opsum)
                for p in range(5, 8):
                    _lru_row(1, p, lpsum)

    nc.finalize()
    _NC_CACHE["nc"] = nc
    return nc


# ---------------------------------------------------------------- entry point

def kernel(**inputs):
    xin, wconv_d, win1, win2, woutm, gsel, vecs = _host_prep(inputs)
    gm = _host_lru_g(inputs)
    nc = _build_nc()

    shared = {
        "wconv": wconv_d,
        "gm": gm,
        "win1": win1,
        "win2": win2,
        "wout": woutm,
        "gsel": gsel,
        "vecs": vecs,
    }
    in_maps = [dict(shared, xin=xin[k]) for k in range(NCORES)]
    res = run_bass_kernel_spmd(nc, in_maps, list(range(NCORES)))

    full = np.zeros((B, C, L, H, W), np.float32)
    for k in range(NCORES):
        yo = res.results[k]["yout"]  # [B, L, C, H, WSL]
        full[:, :, :, :, WSL * k : WSL * k + WSL] = yo.transpose(0, 2, 1, 3, 4)
    return full

